# revision 1
# baseline (speedup 1.0000x reference)
"""Trainium2 Bass kernel for nn_Encoder_82695300317581 (moe_routing).

Data-parallel over batch: each of the 8 NeuronCores processes one image.

Precision plan (routing argmax must match the fp32 reference exactly):
  conv1 + coupler1 in fp32           -> routing1 exact
  switched_conv1 in bf16 hi/lo 3-term compensation (err ~1e-5)
  coupler2 in fp32 on fp32 h2        -> routing2 exact
  selection probs (sel1) in fp32
  switched_conv2 / res blocks in bf16 (value-level noise only, no routing)
"""
import functools

import numpy as np
import ml_dtypes

import concourse.bass as bass
import concourse.tile as tile
from concourse import bacc, mybir
from concourse.bass import ts
from concourse.bass_utils import run_bass_kernel_spmd
from concourse.masks import make_identity

P = 128
N_CORES = 8
F32 = mybir.dt.float32
BF16 = mybir.dt.bfloat16
NEG = 0.01  # leaky relu slope

BF = ml_dtypes.bfloat16


# ---------------------------------------------------------------- host prep

def _im2col76(x_img):
    """x_img [3,256,256] f32 -> [76, 16384] f32 (stride2 pad2 5x5 patches,
    row 75 = ones)."""
    xp = np.pad(x_img, ((0, 0), (2, 2), (2, 2)))
    w = np.lib.stride_tricks.sliding_window_view(xp, (5, 5), axis=(1, 2))[:, ::2, ::2]
    col = w.transpose(0, 3, 4, 1, 2).reshape(75, 128 * 128)
    out = np.empty((76, 128 * 128), np.float32)
    out[:75] = col
    out[75] = 1.0
    return out


def _onehot(dtype):
    oh = np.zeros((8, 8, 128), np.float32)
    for e in range(8):
        oh[e, e, :] = 1.0
    return oh.astype(dtype)


def _prep_weights(w1, b1, c1w, c1b, s1w, s1b, c2w, c2b, s2w, s2b,
                  r0w1, r0b1, r0w2, r0b2, r1w1, r1b1, r1w2, r1b2):
    d = {}
    # conv1 (fp32): [76, 64], row(i*25+ky*5+kx) col(o); row 75 = b1
    w1b = np.zeros((76, 64), np.float32)
    w1b[:75] = w1.transpose(1, 2, 3, 0).reshape(75, 64)
    w1b[75] = b1
    d["w1b"] = w1b
    d["onehot"] = _onehot(BF)
    d["onehotf"] = _onehot(np.float32)
    # coupler1 (fp32): [65, 8]; row 64 = c1b
    c1wb = np.zeros((65, 8), np.float32)
    c1wb[:64] = c1w[:, :, 0, 0].T
    c1wb[64] = c1b
    d["c1wb"] = c1wb
    # sc1 hi (pair-packed): [8, 128, 5, 3, 128]: row j*64+ci -> tap (ky, kx=2f+j)
    whi = s1w.astype(BF).astype(np.float32)     # [o, ci, e, ky, kx]
    wlo = (s1w - whi).astype(np.float32)
    s1wp = np.zeros((8, 2, 64, 5, 3, 128), np.float32)
    for f in range(3):
        for j in range(2):
            kx = 2 * f + j
            if kx <= 4:
                s1wp[:, j, :, :, f, :] = whi[:, :, :, :, kx].transpose(2, 1, 3, 0)
    d["s1wp"] = s1wp.reshape(8, 128, 5, 3, 128).astype(BF)
    # sc1 correction combo: [8, 128, 25, 128]: rows 0:64 = Whi (x h1_lo),
    # rows 64:128 = Wlo (x h1_hi), per single tap t = ky*5+kx
    s1wc = np.zeros((8, 2, 64, 25, 128), np.float32)
    s1wc[:, 0] = whi.transpose(2, 1, 3, 4, 0).reshape(8, 64, 25, 128)
    s1wc[:, 1] = wlo.transpose(2, 1, 3, 4, 0).reshape(8, 64, 25, 128)
    d["s1wc"] = s1wc.reshape(8, 128, 25, 128).astype(BF)
    d["s1b"] = s1b.reshape(128, 1).astype(np.float32)
    # coupler2 (fp32)
    d["c2wf"] = c2w[:, :, 0, 0].T.astype(np.float32).copy()
    d["c2b"] = c2b.reshape(8, 1).astype(np.float32)
    # sc2: [8, 128, 9, 128]
    d["s2w9"] = s2w.transpose(2, 1, 3, 4, 0).reshape(8, 128, 9, 128).astype(BF)
    d["s2b"] = s2b.reshape(128, 1).astype(np.float32)
    for nm, (rw1, rb1, rw2, rb2) in (("r0", (r0w1, r0b1, r0w2, r0b2)),
                                     ("r1", (r1w1, r1b1, r1w2, r1b2))):
        d[nm + "w1t"] = rw1.transpose(1, 2, 3, 0).reshape(128, 9, 32).astype(BF)
        d[nm + "b1"] = rb1.reshape(32, 1).astype(np.float32)
        d[nm + "w2t"] = rw2[:, :, 0, 0].T.astype(BF)
        d[nm + "b2"] = rb2.reshape(128, 1).astype(np.float32)
    return d


# ------------------------------------------------------------- device kernel

def _leaky(nc, pool, out_ap, in_ap, shape):
    """out = max(in, NEG*in); out/in must have identical dim structure."""
    tmp = pool.tile(shape, F32, tag="leaky_tmp")
    tmpv = tmp[:] if len(in_ap.shape) == 2 else \
        tmp[:].rearrange("p (a b) -> p a b", a=in_ap.shape[1])
    nc.vector.tensor_scalar_mul(tmpv, in_ap, NEG)
    nc.vector.tensor_tensor(out_ap, tmpv, in_ap, op=mybir.AluOpType.max)


def _routing(nc, pool, psp, logits_sb, ident_f32, ident_bf, sel_ch, n_px,
             sel_f32):
    """logits_sb [8, n_px] f32 -> sel_ch [8, n_px] (top-1 mask * softmax prob)."""
    n_ch = n_px // P
    lT = pool.tile([P, n_ch, 8], F32, tag="route_lT")
    for c in range(n_ch):
        pt = psp.tile([P, 512], F32, tag="t", name="pst")[:, :8]
        nc.tensor.transpose(pt[:], logits_sb[:, ts(c, P)], ident_f32[:8, :8])
        nc.vector.tensor_copy(lT[:, c, :], pt[:])
    mx = pool.tile([P, n_ch], F32, tag="route_mx")
    nc.vector.tensor_reduce(mx[:], lT[:], axis=mybir.AxisListType.X,
                            op=mybir.AluOpType.max)
    dd = pool.tile([P, n_ch, 8], F32, tag="route_t3", name="dd")
    nc.vector.tensor_tensor(dd[:], lT[:], mx[:, :, None].to_broadcast([P, n_ch, 8]),
                            op=mybir.AluOpType.subtract)
    ee = pool.tile([P, n_ch, 8], F32, tag="route_t3", name="ee")
    nc.scalar.activation(ee[:], dd[:], mybir.ActivationFunctionType.Exp)
    ss = pool.tile([P, n_ch], F32, tag="route_ss")
    nc.vector.tensor_reduce(ss[:], ee[:], axis=mybir.AxisListType.X,
                            op=mybir.AluOpType.add)
    pp = pool.tile([P, n_ch], F32, tag="route_pp")
    nc.vector.reciprocal(pp[:], ss[:])
    mk = pool.tile([P, n_ch, 8], F32, tag="route_t3", name="mk")
    nc.vector.tensor_tensor(mk[:], lT[:], mx[:, :, None].to_broadcast([P, n_ch, 8]),
                            op=mybir.AluOpType.is_equal)
    sdt = F32 if sel_f32 else BF16
    selT = pool.tile([P, n_ch, 8], sdt, tag="route_selT")
    nc.vector.tensor_tensor(selT[:], mk[:], pp[:, :, None].to_broadcast([P, n_ch, 8]),
                            op=mybir.AluOpType.mult)
    for c in range(n_ch):
        if sel_f32:
            pt = psp.tile([P, 512], F32, tag="t", name="psbf")[:8, :P]
            nc.tensor.transpose(pt[:], selT[:, c, :], ident_f32[:])
        else:
            pt = psp.tile([P, 1024], BF16, tag="t", name="psbb")[:8, :P]
            nc.tensor.transpose(pt[:], selT[:, c, :], ident_bf[:])
        nc.vector.tensor_copy(sel_ch[:, ts(c, P)], pt[:])


@functools.lru_cache(maxsize=2)
def build_program(debug=False):
    nc = bacc.Bacc("TRN2", target_bir_lowering=False, debug=False,
                   enable_asserts=False, num_devices=N_CORES)

    def din(name, shape, dt):
        return nc.dram_tensor(name, shape, dt, kind="ExternalInput").ap()

    im2col = din("im2col", [76, 16384], F32)
    w1b = din("w1b", [76, 64], F32)
    c1wb = din("c1wb", [65, 8], F32)
    s1wp = din("s1wp", [8, 128, 5, 3, 128], BF16)
    s1wc = din("s1wc", [8, 128, 25, 128], BF16)
    s1b = din("s1b", [128, 1], F32)
    c2wf = din("c2wf", [128, 8], F32)
    c2b = din("c2b", [8, 1], F32)
    s2w9 = din("s2w9", [8, 128, 9, 128], BF16)
    s2b = din("s2b", [128, 1], F32)
    r0w1t = din("r0w1t", [128, 9, 32], BF16)
    r0b1 = din("r0b1", [32, 1], F32)
    r0w2t = din("r0w2t", [32, 128], BF16)
    r0b2 = din("r0b2", [128, 1], F32)
    r1w1t = din("r1w1t", [128, 9, 32], BF16)
    r1b1 = din("r1b1", [32, 1], F32)
    r1w2t = din("r1w2t", [32, 128], BF16)
    r1b2 = din("r1b2", [128, 1], F32)
    onehot = din("onehot", [8, 8, 128], BF16)
    onehotf = din("onehotf", [8, 8, 128], F32)

    out_ap = nc.dram_tensor("out", [128, 4096], F32, kind="ExternalOutput").ap()
    dbg = {}
    if debug:
        for nm, shp, dt in (("dbg_h1", [128, 132 * 132], BF16),
                            ("dbg_logits1", [8, 4096], F32),
                            ("dbg_sel1", [8, 4096], F32),
                            ("dbg_h2", [128, 4096], F32),
                            ("dbg_logits2", [8, 4096], F32),
                            ("dbg_h3", [128, 4096], F32)):
            dbg[nm] = nc.dram_tensor(nm, shp, dt, kind="ExternalOutput").ap()

    from contextlib import ExitStack
    with tile.TileContext(nc) as tc, ExitStack() as es:
        _build_body(nc, tc, dict(locals(), es=es), dbg)

    nc.compile()
    return nc


def _build_body(nc, tc, t, dbg):
    import os
    KPHASE = int(os.environ.get("KPHASE", "6"))
    im2col, w1b, c1wb, s1wp, s1wc, s1b = (t["im2col"], t["w1b"], t["c1wb"],
                                          t["s1wp"], t["s1wc"], t["s1b"])
    c2wf, c2b, s2w9, s2b = t["c2wf"], t["c2b"], t["s2w9"], t["s2b"]
    rw_aps = {k: t[k] for k in ("r0w1t", "r0w2t", "r1w1t", "r1w2t")}
    out_ap = t["out_ap"]

    es = t["es"]
    big = es.enter_context(tc.tile_pool(name="big", bufs=1))
    pool = es.enter_context(tc.tile_pool(name="work", bufs=2))
    wpool = es.enter_context(tc.tile_pool(name="weights", bufs=2))
    psp = es.enter_context(tc.tile_pool(name="psum", bufs=2, space="PSUM"))
    psy = es.enter_context(tc.tile_pool(name="psum_y", bufs=3, space="PSUM"))
    psb = es.enter_context(tc.tile_pool(name="psum_b", bufs=3, space="PSUM"))

    # constants
    ident_bf = big.tile([P, P], BF16)
    make_identity(nc, ident_bf[:])
    ident_f32 = big.tile([P, P], F32)
    make_identity(nc, ident_f32[:])
    onehot_sb = big.tile([8, 8, P], BF16)
    nc.sync.dma_start(onehot_sb[:], t["onehot"][:])
    onehotf_sb = big.tile([8, 8, P], F32)
    nc.sync.dma_start(onehotf_sb[:], t["onehotf"][:])

    # h1 hi (pair-packed: 0:64 direct, 64:128 x+1-shifted dup)
    h1c = big.tile([P, 132, 132], BF16)
    nc.vector.memset(h1c[:], 0.0)
    # h1 combo for correction: 0:64 = h1_lo, 64:128 = h1_hi (unshifted)
    h1cc = big.tile([P, 132, 132], BF16)
    nc.vector.memset(h1cc[:], 0.0)

    # weights in sbuf
    w1b_sb = big.tile([76, 64], F32)
    nc.sync.dma_start(w1b_sb[:], w1b[:])
    c1wb_sb = big.tile([65, 8], F32)
    nc.sync.dma_start(c1wb_sb[:], c1wb[:])
    c2w_sb = big.tile([P, 8], F32)
    nc.sync.dma_start(c2w_sb[:], c2wf[:])
    small = {}
    for nm, ap_, shp in (("s1b", s1b, [128, 1]), ("c2b", c2b, [8, 1]),
                         ("s2b", s2b, [128, 1]),
                         ("r0b1", t["r0b1"], [32, 1]), ("r0b2", t["r0b2"], [128, 1]),
                         ("r1b1", t["r1b1"], [32, 1]), ("r1b2", t["r1b2"], [128, 1])):
        small[nm] = big.tile(shp, F32, name="cst_" + nm)
        nc.sync.dma_start(small[nm][:], ap_[:])
    rw = {}
    for nm, shp in (("r0w1t", [128, 9, 32]), ("r0w2t", [32, 128]),
                    ("r1w1t", [128, 9, 32]), ("r1w2t", [32, 128])):
        rw[nm] = big.tile(shp, BF16, name="rw_" + nm)
        nc.sync.dma_start(rw[nm][:], rw_aps[nm][:])

    # ---------------- conv1 (fp32) + leaky -> h1 hi/lo + h1s ---------------
    # h1s [65, 4096] f32: leaky'd h1 at even px for coupler1; row 64 = ones
    h1s = big.tile([65, 4096], F32, tag="f4096a", name="h1s")
    nc.vector.memset(h1s[64:65, :], 1.0)
    for nt in range(32):  # y rows 4nt..4nt+3
        imt = wpool.tile([76, 512], F32, tag="wsmall", name="imt")
        nc.sync.dma_start(imt[:], im2col[:, ts(nt, 512)])
        ps = psp.tile([P, 512], F32, tag="t", name="psc1")[:64]
        nc.tensor.matmul(ps[:], lhsT=w1b_sb[:], rhs=imt[:], start=True, stop=True)
        lk = pool.tile([64, 512], F32, tag="c1_lk")
        _leaky(nc, pool, lk[:], ps[:], [64, 512])
        hi = pool.tile([64, 512], BF16, tag="c1_hi")
        nc.vector.tensor_copy(hi[:], lk[:])
        y0 = 4 * nt
        lk4 = lk[:].rearrange("p (a b) -> p a b", b=128)
        hi4 = hi[:].rearrange("p (a b) -> p a b", b=128)
        # hi direct + shifted dup + combo-hi
        nc.vector.tensor_copy(out=h1c[0:64, 2 + y0:2 + y0 + 4, 2:130], in_=hi4)
        nc.vector.tensor_copy(out=h1c[64:128, 2 + y0:2 + y0 + 4, 1:129], in_=hi4)
        nc.vector.tensor_copy(out=h1cc[64:128, 2 + y0:2 + y0 + 4, 2:130], in_=hi4)
        # lo = lk - hi -> combo rows 0:64
        nc.vector.tensor_tensor(h1cc[0:64, 2 + y0:2 + y0 + 4, 2:130], lk4, hi4,
                                op=mybir.AluOpType.subtract)
        # coupler input rows (even y, even x) - both even rows in one copy
        nc.vector.tensor_copy(
            h1s[0:64, ts(nt, 128)].rearrange("p (a b) -> p a b", b=64),
            lk4[:, 0::2, 0::2])

    if dbg:
        nc.sync.dma_start(dbg["dbg_h1"][:], h1c[:].rearrange("p a b -> p (a b)"))
    if KPHASE <= 1:
        ob = big.tile([P, 4096], F32, tag="acc4096", name="ob1")
        nc.vector.memset(ob[:], 0.0)
        nc.sync.dma_start(t["out_ap"][:], ob[:])
        return

    # ---------------- coupler1 (fp32) + routing -> sel1 (fp32) --------------
    logits1 = big.tile([8, 4096], F32, tag="logits", name="logits1")
    sel1 = big.tile([8, 4096], F32, tag="sel", name="sel1")
    for nt in range(8):
        ps = psb.tile([P, 512], F32, tag="b", name="ps8")[:8]
        nc.tensor.matmul(ps[:], lhsT=c1wb_sb[:], rhs=h1s[:, ts(nt, 512)],
                         start=True, stop=True)
        nc.vector.tensor_copy(logits1[:, ts(nt, 512)], ps[:])
    _routing(nc, pool, psp, logits1, ident_f32, ident_bf, sel1, 4096, True)
    if dbg:
        nc.sync.dma_start(dbg["dbg_logits1"][:], logits1[:])
        nc.sync.dma_start(dbg["dbg_sel1"][:], sel1[:])

    if KPHASE <= 2:
        ob = big.tile([P, 4096], F32, tag="acc4096", name="ob2")
        nc.vector.memset(ob[:], 0.0)
        nc.vector.tensor_copy(ob[:8, :], sel1[:])
        nc.sync.dma_start(t["out_ap"][:], ob[:])
        return

    # ---------------- switched conv 1 (dense, hi/lo compensated) -----------
    h2acc = big.tile([P, 4096], F32, tag="acc4096", name="h2acc")
    for e in range(8):
        wt = wpool.tile([P, 15, P], BF16, tag="wsmall", name="wt")
        nc.sync.dma_start(wt[:], s1wp[e].rearrange("k ky f o -> k (ky f) o"))
        wtc = wpool.tile([P, 25, P], BF16, tag="wbig", name="wtc")
        nc.sync.dma_start(wtc[:], s1wc[e])
        for nt in range(8):  # h rows 8nt..8nt+7, w 0..63
            ps = psy.tile([P, 512], F32, tag="y", name="psy1")
            h0 = 8 * nt
            # main term: pair-packed hi x hi
            for ky in range(5):
                for f in range(3):
                    rhs = h1c[:, 2 * h0 + ky:2 * h0 + ky + 16:2,
                              2 * f:2 * f + 128:2]
                    nc.tensor.matmul(ps[:], lhsT=wt[:, ky * 3 + f, :], rhs=rhs,
                                     start=(ky == 0 and f == 0), stop=False)
            # correction: per-tap combo (Whi x h_lo + Wlo x h_hi)
            for tap in range(25):
                ky, kx = tap // 5, tap % 5
                rhs = h1cc[:, 2 * h0 + ky:2 * h0 + ky + 16:2, kx:kx + 128:2]
                nc.tensor.matmul(ps[:], lhsT=wtc[:, tap, :], rhs=rhs,
                                 start=False, stop=(tap == 24))
            bc = psb.tile([P, 512], F32, tag="b", name="psbc1")
            nc.tensor.matmul(bc[:], lhsT=onehotf_sb[:, e, :],
                             rhs=sel1[:, ts(nt, 512)], start=True, stop=True)
            bcs = pool.tile([P, 512], F32, tag="leaky_tmp", name="bcs")
            nc.vector.tensor_copy(bcs[:], bc[:])
            if e == 0:
                nc.vector.tensor_tensor(h2acc[:, ts(nt, 512)], ps[:], bcs[:],
                                        op=mybir.AluOpType.mult)
            else:
                tmp = pool.tile([P, 512], F32, tag="cmb")
                nc.vector.tensor_tensor(tmp[:], ps[:], bcs[:],
                                        op=mybir.AluOpType.mult)
                nc.vector.tensor_tensor(h2acc[:, ts(nt, 512)],
                                        h2acc[:, ts(nt, 512)], tmp[:],
                                        op=mybir.AluOpType.add)

    if KPHASE <= 3:
        nc.sync.dma_start(t["out_ap"][:], h2acc[:])
        return

    # h2f = leaky(h2acc + s1b) fp32; h2c = bf16(h2f) padded
    h2f = big.tile([P, 4096], F32, tag="f4096a", name="h2f")
    h2c = big.tile([P, 66, 66], BF16, tag="pad66", name="h2c")
    nc.vector.memset(h2c[:], 0.0)
    for nt in range(8):
        xb = pool.tile([P, 512], F32, tag="h2xb")
        nc.vector.tensor_scalar_add(xb[:], h2acc[:, ts(nt, 512)], small["s1b"][:])
        _leaky(nc, pool, h2f[:, ts(nt, 512)], xb[:], [P, 512])
        nc.scalar.activation(
            h2c[:, 1 + 8 * nt:1 + 8 * nt + 8, 1:65],
            h2f[:, ts(nt, 512)].rearrange("p (a b) -> p a b", b=64),
            mybir.ActivationFunctionType.Copy)
    if dbg:
        nc.sync.dma_start(dbg["dbg_h2"][:], h2f[:])

    if KPHASE <= 4:
        nc.sync.dma_start(t["out_ap"][:], h2f[:])
        return

    # ---------------- coupler2 (fp32) + routing -> sel2 (bf16) -------------
    logits2 = big.tile([8, 4096], F32, tag="logits", name="logits2")
    sel2 = big.tile([8, 4096], BF16, tag="sel", name="sel2")
    for nt in range(8):
        ps = psb.tile([P, 512], F32, tag="b", name="ps8b")[:8]
        nc.tensor.matmul(ps[:], lhsT=c2w_sb[:], rhs=h2f[:, ts(nt, 512)],
                         start=True, stop=True)
        nc.vector.tensor_scalar_add(logits2[:, ts(nt, 512)], ps[:],
                                    small["c2b"][:])
    _routing(nc, pool, psp, logits2, ident_f32, ident_bf, sel2, 4096, False)
    if dbg:
        nc.sync.dma_start(dbg["dbg_logits2"][:], logits2[:])

    # ---------------- switched conv 2 (dense bf16) -------------------------
    h3acc = big.tile([P, 4096], F32, tag="acc4096", name="h3acc")
    for e in range(8):
        wt2 = wpool.tile([P, 25, P], BF16, tag="wbig", name="wt2")[:, :9, :]
        nc.sync.dma_start(wt2[:], s2w9[e])
        for nt in range(8):
            ps = psy.tile([P, 512], F32, tag="y", name="psy2")
            h0 = 8 * nt
            for tap in range(9):
                ky, kx = tap // 3, tap % 3
                rhs = h2c[:, h0 + ky:h0 + ky + 8, kx:kx + 64]
                nc.tensor.matmul(ps[:], lhsT=wt2[:, tap, :], rhs=rhs,
                                 start=(tap == 0), stop=(tap == 8))
            bc = psb.tile([P, 512], F32, tag="b", name="psbc2")
            nc.tensor.matmul(bc[:], lhsT=onehot_sb[:, e, :],
                             rhs=sel2[:, ts(nt, 512)], start=True, stop=True)
            bcs = pool.tile([P, 512], F32, tag="leaky_tmp", name="bcs")
            nc.vector.tensor_copy(bcs[:], bc[:])
            if e == 0:
                nc.vector.tensor_tensor(h3acc[:, ts(nt, 512)], ps[:], bcs[:],
                                        op=mybir.AluOpType.mult)
            else:
                tmp = pool.tile([P, 512], F32, tag="cmb")
                nc.vector.tensor_tensor(tmp[:], ps[:], bcs[:],
                                        op=mybir.AluOpType.mult)
                nc.vector.tensor_tensor(h3acc[:, ts(nt, 512)],
                                        h3acc[:, ts(nt, 512)], tmp[:],
                                        op=mybir.AluOpType.add)

    if KPHASE <= 5:
        nc.sync.dma_start(t["out_ap"][:], h3acc[:])
        return

    # h3 = h3acc + s2b -> h3c (f32 padded); h3r = relu(h3) bf16
    h3c = big.tile([P, 66, 66], BF16, name="h3c")
    nc.vector.memset(h3c[:], 0.0)
    h3r = big.tile([P, 66, 66], BF16, tag="pad66", name="h3r")
    nc.vector.memset(h3r[:], 0.0)
    for nt in range(8):
        dst = h3c[:, 1 + 8 * nt:1 + 8 * nt + 8, 1:65]
        nc.vector.tensor_scalar_add(
            dst, h3acc[:, ts(nt, 512)].rearrange("p (a b) -> p a b", b=64),
            small["s2b"][:])
        nc.scalar.activation(h3r[:, 1 + 8 * nt:1 + 8 * nt + 8, 1:65],
                             dst, mybir.ActivationFunctionType.Relu)
    if dbg:
        h3d = pool.tile([P, 512], F32, tag="h2xb", name="h3d")
        for nt in range(8):
            nc.vector.tensor_copy(
                h3d[:].rearrange("p (a b) -> p a b", b=64),
                h3c[:, 1 + 8 * nt:1 + 8 * nt + 8, 1:65])
            nc.sync.dma_start(dbg["dbg_h3"][:, ts(nt, 512)], h3d[:])

    # ---------------- res blocks ------------------------------------------
    t1 = big.tile([32, 4096], BF16, tag="sel", name="t1")
    out_sb = big.tile([P, 4096], F32, tag="acc4096", name="out_sb")
    for rn, (w1t_, b1_, w2t_, b2_) in (("r0", ("r0w1t", "r0b1", "r0w2t", "r0b2")),
                                       ("r1", ("r1w1t", "r1b1", "r1w2t", "r1b2"))):
        for nt in range(8):
            ps = psy.tile([P, 512], F32, tag="y", name="ps32")[:32]
            h0 = 8 * nt
            for tap in range(9):
                ky, kx = tap // 3, tap % 3
                rhs = h3r[:, h0 + ky:h0 + ky + 8, kx:kx + 64]
                nc.tensor.matmul(ps[:], lhsT=rw[w1t_][:, tap, :], rhs=rhs,
                                 start=(tap == 0), stop=(tap == 8))
            nc.scalar.activation(t1[:, ts(nt, 512)], ps[:],
                                 mybir.ActivationFunctionType.Relu,
                                 bias=small[b1_][:])
        for nt in range(8):
            ps = psy.tile([P, 512], F32, tag="y", name="psd")
            nc.tensor.matmul(ps[:], lhsT=rw[w2t_][:], rhs=t1[:, ts(nt, 512)],
                             start=True, stop=True)
            tmp = pool.tile([P, 512], F32, tag="res_add")
            nc.vector.tensor_scalar_add(tmp[:], ps[:], small[b2_][:])
            dst = h3c[:, 1 + 8 * nt:1 + 8 * nt + 8, 1:65]
            nc.vector.tensor_tensor(
                dst, dst, tmp[:].rearrange("p (a b) -> p a b", b=64),
                op=mybir.AluOpType.add)
            if rn == "r0":  # refresh relu'd copy for res1
                nc.scalar.activation(h3r[:, 1 + 8 * nt:1 + 8 * nt + 8, 1:65],
                                     dst, mybir.ActivationFunctionType.Relu)

    # ---------------- final leaky -> out ----------------------------------
    for nt in range(8):
        sq = h3c[:, 1 + 8 * nt:1 + 8 * nt + 8, 1:65]
        _leaky(nc, pool,
               out_sb[:, ts(nt, 512)].rearrange("p (a b) -> p a b", b=64),
               sq, [P, 512])
    nc.sync.dma_start(out_ap[:], out_sb[:])


# ----------------------------------------------------------------- entry

def _in_maps(inputs):
    x = np.asarray(inputs["x"], np.float32)
    wd = _prep_weights(**{k: np.asarray(v, np.float32) for k, v in inputs.items()
                          if k != "x"})
    maps = []
    for c in range(N_CORES):
        m = dict(wd)
        m["im2col"] = _im2col76(x[c])
        maps.append(m)
    return maps


def kernel(**inputs):
    nc = build_program(False)
    res = run_bass_kernel_spmd(nc, _in_maps(inputs), core_ids=list(range(N_CORES)),
                               trace=False)
    out = np.stack([res.results[c]["out"].reshape(128, 64, 64)
                    for c in range(N_CORES)])
    return out.astype(np.float32)


def run_debug(inputs):
    nc = build_program(True)
    res = run_bass_kernel_spmd(nc, _in_maps(inputs), core_ids=list(range(N_CORES)),
                               trace=False)
    out = np.stack([res.results[c]["out"].reshape(128, 64, 64)
                    for c in range(N_CORES)])
    return out.astype(np.float32), res.results

